# revision 9
# baseline (speedup 1.0000x reference)
"""Trainium2 Bass kernel for nn_MoELayer (moe_routing).

Reference computation (B=8192 tokens, d=1024, E=8 experts, top-k=2):
    gating  = softmax(x @ gate_w + gate_b)                    # [B, E]
    mask    = top-2 one-hot scatter of gating                 # [B, E]
    blockm  = mask.reshape(B//d, d, E).max(axis=1)            # per 1024-row block
    out     = sum_e gating[:, e] * blockm[block(b), e] * (x @ W[:, e*d:(e+1)*d])

Structural facts exploited:
  * The combine uses the FULL softmax weights; the top-2 mask enters only
    through the per-1024-row-block max, which is all-ones w.h.p. at these
    sizes. The compute is dense: out = sum_e (gating*blockmask) .* (x @ W_e).
  * Gating and block mask for a 1024-row block depend only on that block.

Sharding: data-parallel over the 8 row blocks of 1024 tokens (one per core).
No cross-core communication; W is streamed (replicated) to every core.

Precision/engine strategy (v2):
  * Main matmuls run in float32r (fp32 operands read at 1 row/cycle, ~fp22
    precision). x and W tiles are PE-ready straight off DMA -- no bf16
    conversion passes on DVE, which both removes ~35us of DVE work and takes
    the conversions off the startup critical path.
  * For experts in FP8_EXPERTS, the last two k-tiles (k=6,7) are computed by
    a single fp8e4 DoubleRow matmul (K=256 per instruction, 2x FLOP rate).
    Scales are balanced (x/16, W*16) so the fp8 product lands at scale 1.0
    and accumulates into the same PSUM group as the fp32r matmuls.
    Quantization error is confined to 2/8 of the contraction on those
    experts; with 6 experts the end-to-end rel err is ~1.7e-2 (< 2e-2 gate,
    deterministic inputs).
  * Per-core schedule follows the v1 baseline: expert 0 is computed UNSCALED
    in two k-half passes starting as soon as the first tiles land (no gating
    dependency); gating (fp32r logit matmuls, softmax, top-2 mask, block
    mask) runs between; expert 1 is also computed unscaled; both are merged
    into acc with their gating scales during experts 3-6. ACT applies the
    per-token gating scale PSUM->SBUF; DVE accumulates.
  * Output DMA for each m-tile issues as soon as expert 7's combine for that
    tile completes, so the store overlaps the tail.
"""

import numpy as np

P = 128          # partitions
D = 1024         # d_model
E = 8            # experts
TOK = 1024       # tokens per core (row block)
KT = D // P      # contraction tiles
KH = KT // 2     # k-half for expert 0's two passes
MT = TOK // P    # token tiles
NH = 512         # psum half-width (one fp32 bank)
N_CORES = 8
WARMUP_MMS = 8

# Experts whose last k-pair (k=6,7) is computed with one fp8e4 DoubleRow
# matmul instead of two fp32r matmuls. More experts -> faster but larger
# quantization error. 7 experts measures ~1.8e-2 rel err vs the 2e-2 gate.
FP8_EXPERTS = frozenset((1, 2, 3, 4, 5, 6, 7))
SX = 1.0 / 16.0  # x fp8 encode scale
SW = 16.0        # W fp8 encode scale (SX*SW == 1 -> products at scale 1)


def _build_nc():
    import concourse.bacc as bacc
    import concourse.mybir as mybir
    import concourse.tile as tile

    f32 = mybir.dt.float32
    f32r = mybir.dt.float32r
    bf16 = mybir.dt.bfloat16
    f8 = mybir.dt.float8e4
    AX = mybir.AxisListType
    OP = mybir.AluOpType
    AF = mybir.ActivationFunctionType
    DR = mybir.MatmulPerfMode.DoubleRow

    nc = bacc.Bacc(None, target_bir_lowering=False, debug=False)
    xT_d = nc.dram_tensor("xT", [D, TOK], f32r, kind="ExternalInput")
    w_d = nc.dram_tensor("w", [D, E * D], f32r, kind="ExternalInput")
    gw_d = nc.dram_tensor("gate_w", [D, E], f32r, kind="ExternalInput")
    gb_d = nc.dram_tensor("gate_b", [1, E], f32, kind="ExternalInput")
    out_d = nc.dram_tensor("out", [TOK, D], f32, kind="ExternalOutput")

    xT_r = xT_d.rearrange("(k p) t -> k p t", p=P)
    w_r = w_d.rearrange("(k p) (e f) -> k p e f", p=P, f=D)
    gw_r = gw_d.rearrange("(k p) e -> p k e", p=P)
    out_r = out_d.rearrange("(m p) f -> m p f", p=P)

    def v32(ap):  # f32r SBUF data viewed as plain f32 for DVE/ACT reads
        return ap.bitcast(f32)

    with tile.TileContext(nc) as tc:
        with (
            tc.tile_pool(name="persist", bufs=1) as persist,
            tc.tile_pool(name="gstat", bufs=2) as p_gs,
            tc.tile_pool(name="wf", bufs=2 * KT) as p_wf,
            tc.tile_pool(name="tmp", bufs=6) as p_tmp,
            tc.tile_pool(name="ps_gate", bufs=1, space="PSUM") as ps_gate,
            tc.tile_pool(name="ps_cnt", bufs=1, space="PSUM") as ps_cnt,
            tc.tile_pool(name="ps_mm", bufs=6, space="PSUM") as ps_mm,
        ):
            # -- front matter (no DMA dependencies)
            wu_lhs = persist.tile([P, P], bf16, tag="wu_lhs")
            nc.vector.memset(wu_lhs[:], 0.0)
            wu_rhs = persist.tile([P, NH], bf16, tag="wu_rhs")
            nc.vector.memset(wu_rhs[:], 0.0)
            ones_col = persist.tile([P, 1], bf16, tag="ones_col")
            nc.vector.memset(ones_col[:], 1.0)
            exp_in = persist.tile([P, 1], f32, tag="exp_in")
            nc.vector.memset(exp_in[:], 1.0)
            ones_row_bf = persist.tile([1, P], bf16, tag="ones_row_bf")
            nc.vector.memset(ones_row_bf[:], 1.0)
            id8_i = persist.tile([E, E], mybir.dt.int32, tag="id8_i")
            nc.gpsimd.iota(id8_i[:], pattern=[[1, E]], base=0,
                           channel_multiplier=-1)
            id8 = persist.tile([E, E], bf16, tag="id8")
            nc.vector.tensor_scalar(id8[:], id8_i[:], 0, None, op0=OP.is_equal)
            exp_dummy = persist.tile([1, 1], f32, tag="exp_dummy")
            nc.scalar.activation(exp_dummy[:], exp_in[:1, :], AF.Exp)

            # PE warm-up while the first DMAs land
            wu_ps = ps_cnt.tile([P, NH], f32, tag="cnt")
            for i in range(WARMUP_MMS):
                nc.tensor.matmul(
                    wu_ps[:], wu_lhs[:], wu_rhs[:],
                    start=(i == 0), stop=(i == WARMUP_MMS - 1),
                )

            # -- loads: x^T k-tiles interleaved with expert-0 weight k-tiles.
            # fp32 tiles are the PE operands directly (fp32r) -- usable the
            # moment the DMA lands.
            xf = [persist.tile([P, TOK], f32r, tag=f"xf{k}", name=f"xf{k}")
                  for k in range(KT)]
            wf0 = [p_wf.tile([P, D], f32r, tag="wf", name=f"wf0_{k}")
                   for k in range(KT)]
            # interleaved (x_k, w0_k) pairs keep both the compute and the
            # weight stream fed; k0-3 go in half-tile chunks so the first
            # matmuls start as early as possible.
            for k in range(KT):
                if k < KH:
                    for h in range(2):
                        nc.sync.dma_start(xf[k][:, h * NH:(h + 1) * NH],
                                          xT_r[k][:, h * NH:(h + 1) * NH])
                        nc.sync.dma_start(wf0[k][:, h * NH:(h + 1) * NH],
                                          w_r[k, :, 0, h * NH:(h + 1) * NH])
                else:
                    nc.sync.dma_start(xf[k][:], xT_r[k])
                    nc.sync.dma_start(wf0[k][:], w_r[k, :, 0, :])

            gb_col = persist.tile([E, 1], f32, tag="gb_col")
            nc.sync.dma_start(gb_col[:], gb_d.rearrange("o e -> e o"))
            bmb = persist.tile([P, E], f32, tag="bmb")
            gw_in = persist.tile([P, KT, E], f32r, tag="gw_in")
            nc.sync.dma_start(gw_in[:], gw_r[:])

            # fp8 operand for the DoubleRow k-pair: x8[:, i, t] = (x/16) for
            # k-tile 6+i, all 1024 tokens. lhsT slices are [p, 2, 128].
            x8 = persist.tile([P, 2, TOK], f8, tag="x8")
            for i in range(2):
                nc.vector.tensor_scalar(x8[:, i, :], v32(xf[6 + i][:]), SX,
                                        None, op0=OP.mult)

            # acc: fp32 output accumulator. acc0/acc1: experts 0/1 computed
            # unscaled (bf16 storage), merged with their gating scales later.
            acc = []
            acc0 = []
            for m in range(MT):
                acc.append(persist.tile([P, D], f32, tag=f"acc{m}",
                                        name=f"acc{m}"))
                acc0.append(persist.tile([P, D], bf16, tag=f"acc0{m}",
                                         name=f"acc0{m}"))

            fill_ps = ps_gate.tile([P, NH], f32, tag="bmb_ps", bufs=1)

            def pe_filler(n=1):
                for _ in range(n):
                    nc.tensor.matmul(fill_ps[:, :NH], wu_lhs[:], wu_rhs[:],
                                     start=True, stop=True)

            # -- expert 0, pass A (k = 0..3), unscaled -> acc0
            for m in range(MT):
                ps0 = ps_mm.tile([P, NH], f32, tag="psmm")
                ps1 = ps_mm.tile([P, NH], f32, tag="psmm")
                for k in range(KH):
                    lhs = xf[k][:, m * P:(m + 1) * P]
                    nc.tensor.matmul(ps0[:], lhs, wf0[k][:, 0:NH],
                                     start=(k == 0), stop=(k == KH - 1))
                    nc.tensor.matmul(ps1[:], lhs, wf0[k][:, NH:D],
                                     start=(k == 0), stop=(k == KH - 1))
                    if m == 0:
                        pe_filler(2)
                nc.scalar.copy(acc0[m][:, 0:NH], ps0[:])
                nc.scalar.copy(acc0[m][:, NH:D], ps1[:])

            def load_w(e):
                tiles = []
                for k in range(KT):
                    wt = p_wf.tile([P, D], f32r, tag="wf", name=f"wf{e}_{k}")
                    nc.sync.dma_start(wt[:], w_r[k, :, e, :])
                    tiles.append(wt)
                return tiles

            def quant_w(e, wtiles):
                """fp8 W k-pair for expert e: w8[:, i, :] = W[k=6+i]*16."""
                w8 = p_tmp.tile([P, 2, D], f8, tag="w8", name=f"w8_{e}",
                                bufs=2)
                for i in range(2):
                    nc.vector.tensor_scalar(w8[:, i, :],
                                            v32(wtiles[6 + i][:]),
                                            SW, None, op0=OP.mult)
                return w8

            wf_cur = load_w(1)
            w8_cur = quant_w(1, wf_cur) if 1 in FP8_EXPERTS else None

            # -- gating logits, TRANSPOSED ([E, tokens]); fp32r from raw x
            lgT0 = ps_mm.tile([E, NH], f32, tag="psmm")
            lgT1 = ps_mm.tile([E, NH], f32, tag="psmm")
            for k in range(KT):
                nc.tensor.matmul(lgT0[:], gw_in[:, k, :], xf[k][:, 0:NH],
                                 start=(k == 0), stop=(k == KT - 1))
                nc.tensor.matmul(lgT1[:], gw_in[:, k, :], xf[k][:, NH:TOK],
                                 start=(k == 0), stop=(k == KT - 1))
            # + gate_b: per-partition bias in the transposed [E, tokens] layout
            lgT_sb = persist.tile([E, TOK], bf16, tag="lgT_sb")
            nc.scalar.activation(lgT_sb[:, 0:NH], lgT0[:], AF.Identity,
                                 bias=gb_col[:])
            nc.scalar.activation(lgT_sb[:, NH:TOK], lgT1[:], AF.Identity,
                                 bias=gb_col[:])

            # -- expert 0, pass B (k = 4..7), accumulate into acc0 on DVE
            w80 = quant_w(0, wf0) if 0 in FP8_EXPERTS else None
            for m in range(MT):
                ps0 = ps_mm.tile([P, NH], f32, tag="psmm")
                ps1 = ps_mm.tile([P, NH], f32, tag="psmm")
                if 0 in FP8_EXPERTS:
                    for k in range(KH, 6):
                        lhs = xf[k][:, m * P:(m + 1) * P]
                        nc.tensor.matmul(ps0[:], lhs, wf0[k][:, 0:NH],
                                         start=(k == KH), stop=False)
                        nc.tensor.matmul(ps1[:], lhs, wf0[k][:, NH:D],
                                         start=(k == KH), stop=False)
                    lhs8 = x8[:, :, m * P:(m + 1) * P]
                    nc.tensor.matmul(ps0[:], lhs8, w80[:, :, 0:NH],
                                     start=False, stop=True, perf_mode=DR)
                    nc.tensor.matmul(ps1[:], lhs8, w80[:, :, NH:D],
                                     start=False, stop=True, perf_mode=DR)
                else:
                    for k in range(KH, KT):
                        lhs = xf[k][:, m * P:(m + 1) * P]
                        nc.tensor.matmul(ps0[:], lhs, wf0[k][:, 0:NH],
                                         start=(k == KH), stop=(k == KT - 1))
                        nc.tensor.matmul(ps1[:], lhs, wf0[k][:, NH:D],
                                         start=(k == KH), stop=(k == KT - 1))
                nc.vector.tensor_tensor(acc0[m][:, 0:NH], acc0[m][:, 0:NH],
                                        ps0[:], op=OP.add)
                nc.vector.tensor_tensor(acc0[m][:, NH:D], acc0[m][:, NH:D],
                                        ps1[:], op=OP.add)

            # Gating part 2: transpose via K=8 matmul against identity, then
            # softmax + top-2 mask; masks packed for a single count matmul.
            mask_all = persist.tile([P, MT * E], bf16, tag="mask_all")
            gfin = []
            gsc = [persist.tile([P, E], f32, tag=f"gsc{m}", name=f"gsc{m}")
                   for m in range(MT)]
            for m in range(MT):
                lg = ps_cnt.tile([P, E], f32, tag="cnt", bufs=1)
                nc.tensor.matmul(lg[:], lgT_sb[:, m * P:(m + 1) * P], id8[:],
                                 start=True, stop=True)
                ex = p_gs.tile([P, E], f32, tag="ex")
                nc.scalar.activation(ex[:], lg[:], AF.Exp)
                ssum = p_gs.tile([P, 1], f32, tag="ssum")
                nc.vector.reduce_sum(ssum[:], ex[:], axis=AX.X)
                rcp = p_gs.tile([P, 1], f32, tag="rcp")
                nc.vector.reciprocal(rcp[:], ssum[:])
                m1 = p_gs.tile([P, 1], f32, tag="m1")
                nc.vector.reduce_max(m1[:], ex[:], axis=AX.X)
                eqb = p_gs.tile([P, E], f32, tag="eqb")
                nc.vector.tensor_scalar(
                    eqb[:], ex[:], m1[:], -1e30, op0=OP.is_ge, op1=OP.mult
                )
                g2 = p_gs.tile([P, E], f32, tag="g2")
                nc.vector.tensor_tensor(g2[:], ex[:], eqb[:], op=OP.add)
                m2 = p_gs.tile([P, 1], f32, tag="m2")
                nc.vector.reduce_max(m2[:], g2[:], axis=AX.X)
                nc.vector.tensor_scalar(mask_all[:, m * E:(m + 1) * E],
                                        ex[:], m2[:], None, op0=OP.is_ge)
                gt = p_gs.tile([P, E], f32, tag=f"gt{m}", bufs=1)
                nc.vector.tensor_scalar_mul(gt[:], ex[:], rcp[:])
                gfin.append(gt)

            cnt_ps = ps_cnt.tile([1, MT * E], f32, tag="cnt")
            nc.tensor.matmul(cnt_ps[:], ones_col[:], mask_all[:],
                             start=True, stop=True)
            cnt_sb = p_gs.tile([1, MT * E], f32, tag="cnt_sb")
            nc.vector.tensor_copy(cnt_sb[:], cnt_ps[:])
            cnt_e = p_gs.tile([1, E], f32, tag="cnt_e")
            nc.vector.tensor_reduce(
                cnt_e[:], cnt_sb[:].rearrange("p (m e) -> p e m", e=E),
                axis=AX.X, op=OP.add,
            )
            bm01 = p_gs.tile([1, E], bf16, tag="bm01")
            nc.vector.tensor_scalar(bm01[:], cnt_e[:], 0.5, None, op0=OP.is_ge)
            bmb_ps = ps_gate.tile([P, E], f32, tag="bmb_ps", bufs=1)
            nc.tensor.matmul(bmb_ps[:], ones_row_bf[:], bm01[:],
                             start=True, stop=True)
            nc.vector.tensor_copy(bmb[:], bmb_ps[:])
            for m in range(MT):
                nc.vector.tensor_tensor(gsc[m][:], gfin[m][:], bmb[:],
                                        op=OP.mult)

            # -- experts 1..7
            for e in range(1, E):
                wfe = wf_cur
                w8e = w8_cur
                if e + 1 < E:
                    wf_cur = load_w(e + 1)
                    w8_cur = (quant_w(e + 1, wf_cur)
                              if (e + 1) in FP8_EXPERTS else None)
                use8 = e in FP8_EXPERTS
                kmax = 6 if use8 else KT
                for m in range(MT):
                    ps0 = ps_mm.tile([P, NH], f32, tag="psmm")
                    ps1 = ps_mm.tile([P, NH], f32, tag="psmm")
                    for k in range(kmax):
                        lhs = xf[k][:, m * P:(m + 1) * P]
                        nc.tensor.matmul(ps0[:], lhs, wfe[k][:, 0:NH],
                                         start=(k == 0),
                                         stop=(not use8 and k == KT - 1))
                        nc.tensor.matmul(ps1[:], lhs, wfe[k][:, NH:D],
                                         start=(k == 0),
                                         stop=(not use8 and k == KT - 1))
                    if use8:
                        lhs8 = x8[:, :, m * P:(m + 1) * P]
                        nc.tensor.matmul(ps0[:], lhs8, w8e[:, :, 0:NH],
                                         start=False, stop=True, perf_mode=DR)
                        nc.tensor.matmul(ps1[:], lhs8, w8e[:, :, NH:D],
                                         start=False, stop=True, perf_mode=DR)
                    for h, ps in ((0, ps0), (1, ps1)):
                        osl = acc[m][:, h * NH:(h + 1) * NH]
                        if e == 1:
                            # gating is ready by expert 1's drains here, so
                            # expert 1 initializes acc directly scaled
                            nc.scalar.mul(osl, ps[:], gsc[m][:, e:e + 1])
                        else:
                            tmp = p_tmp.tile([P, NH], f32, tag="tmp")
                            nc.scalar.mul(tmp[:], ps[:], gsc[m][:, e:e + 1])
                            nc.vector.tensor_tensor(osl, osl, tmp[:],
                                                    op=OP.add)
                    if e in (3, 4, 5, 6) and m // 2 == e - 3:
                        # fold in g0 * acc0, two m-tiles per expert; mul on
                        # DVE to keep ACT off the critical path
                        gcol = gsc[m][:, 0:1]
                        for h in range(2):
                            osl = acc[m][:, h * NH:(h + 1) * NH]
                            asl = acc0[m][:, h * NH:(h + 1) * NH]
                            tmp = p_tmp.tile([P, NH], f32, tag="tmp")
                            nc.vector.tensor_scalar_mul(tmp[:], asl, gcol)
                            nc.vector.tensor_tensor(osl, osl, tmp[:],
                                                    op=OP.add)
                    if e == E - 1:
                        nc.sync.dma_start(out_r[m][:, 0:NH], acc[m][:, 0:NH])
                        nc.sync.dma_start(out_r[m][:, NH:D], acc[m][:, NH:D])

    nc.compile()
    return nc


def _ensure_ntff_hook_module():
    """Defensive: some environments lack ``antenv.axon_hooks``; if a caller
    sets BASS_TRACE=1, run_bass_kernel_spmd imports it unconditionally and
    would crash. Provide a working shim."""
    import sys
    import types

    try:
        import antenv.axon_hooks  # noqa: F401
        return
    except ImportError:
        pass
    try:
        import antenv  # noqa: F401
    except ImportError:
        return
    m = types.ModuleType("antenv.axon_hooks")
    exec(
        "_hook = None\n"
        "def set_axon_ntff_profile_hook(h):\n"
        "    global _hook\n"
        "    _hook = h\n"
        "def get_axon_ntff_profile_hook():\n"
        "    return _hook\n",
        m.__dict__,
    )
    sys.modules["antenv.axon_hooks"] = m
    try:
        from trn_agent_boot.trn_boot import _ntff_profile_via_ctypes

        m.set_axon_ntff_profile_hook(
            _ntff_profile_via_ctypes("/opt/axon/libaxon_pjrt.so")
        )
    except Exception:
        pass


_ensure_ntff_hook_module()

_CACHE = {}
LAST_RESULTS = None  # BassKernelResults of the most recent run (for test.py)


def _get_nc():
    if "nc" not in _CACHE:
        _CACHE["nc"] = _build_nc()
    return _CACHE["nc"]


def kernel(x, W, gate_w, gate_b):
    global LAST_RESULTS
    from concourse.bass_utils import run_bass_kernel_spmd

    x = np.ascontiguousarray(np.asarray(x, dtype=np.float32))
    W = np.ascontiguousarray(np.asarray(W, dtype=np.float32))
    gate_w = np.ascontiguousarray(np.asarray(gate_w, dtype=np.float32))
    gb = np.ascontiguousarray(np.asarray(gate_b, dtype=np.float32).reshape(1, E))

    in_maps = []
    for c in range(N_CORES):
        xT = np.ascontiguousarray(x[c * TOK:(c + 1) * TOK].T)
        in_maps.append({"xT": xT, "w": W, "gate_w": gate_w, "gate_b": gb})

    res = run_bass_kernel_spmd(_get_nc(), in_maps, core_ids=list(range(N_CORES)))
    LAST_RESULTS = res
    return np.concatenate([r["out"] for r in res.results], axis=0)


# revision 11
# speedup vs baseline: 1.0097x; 1.0097x over previous
"""Trainium2 Bass kernel for nn_MoELayer (moe_routing).

Reference computation (B=8192 tokens, d=1024, E=8 experts, top-k=2):
    gating  = softmax(x @ gate_w + gate_b)                    # [B, E]
    mask    = top-2 one-hot scatter of gating                 # [B, E]
    blockm  = mask.reshape(B//d, d, E).max(axis=1)            # per 1024-row block
    out     = sum_e gating[:, e] * blockm[block(b), e] * (x @ W[:, e*d:(e+1)*d])

Structural facts exploited:
  * The combine uses the FULL softmax weights; the top-2 mask enters only
    through the per-1024-row-block max, which is all-ones w.h.p. at these
    sizes. The compute is dense: out = sum_e (gating*blockmask) .* (x @ W_e).
  * Gating and block mask for a 1024-row block depend only on that block.

Sharding: data-parallel over the 8 row blocks of 1024 tokens (one per core).
No cross-core communication; W is streamed (replicated) to every core.

Precision/engine strategy (v2):
  * Main matmuls run in float32r (fp32 operands read at 1 row/cycle, ~fp22
    precision). x and W tiles are PE-ready straight off DMA -- no bf16
    conversion passes on DVE, which both removes ~35us of DVE work and takes
    the conversions off the startup critical path.
  * For experts in FP8_EXPERTS, the last two k-tiles (k=6,7) are computed by
    a single fp8e4 DoubleRow matmul (K=256 per instruction, 2x FLOP rate).
    Scales are balanced (x/16, W*16) so the fp8 product lands at scale 1.0
    and accumulates into the same PSUM group as the fp32r matmuls.
    Quantization error is confined to 2/8 of the contraction on those
    experts; with 6 experts the end-to-end rel err is ~1.7e-2 (< 2e-2 gate,
    deterministic inputs).
  * Per-core schedule follows the v1 baseline: expert 0 is computed UNSCALED
    in two k-half passes starting as soon as the first tiles land (no gating
    dependency); gating (fp32r logit matmuls, softmax, top-2 mask, block
    mask) runs between; expert 1 is also computed unscaled; both are merged
    into acc with their gating scales during experts 3-6. ACT applies the
    per-token gating scale PSUM->SBUF; DVE accumulates.
  * Output DMA for each m-tile issues as soon as expert 7's combine for that
    tile completes, so the store overlaps the tail.
"""

import numpy as np

P = 128          # partitions
D = 1024         # d_model
E = 8            # experts
TOK = 1024       # tokens per core (row block)
KT = D // P      # contraction tiles
KH = KT // 2     # k-half for expert 0's two passes
MT = TOK // P    # token tiles
NH = 512         # psum half-width (one fp32 bank)
N_CORES = 8
WARMUP_MMS = 8

# Experts whose last k-pair (k=6,7) is computed with one fp8e4 DoubleRow
# matmul instead of two fp32r matmuls. More experts -> faster but larger
# quantization error. 7 experts measures ~1.8e-2 rel err vs the 2e-2 gate.
FP8_EXPERTS = frozenset((1, 2, 3, 4, 5, 6, 7))
SX = 1.0 / 16.0  # x fp8 encode scale
SW = 16.0        # W fp8 encode scale (SX*SW == 1 -> products at scale 1)


def _build_nc():
    import concourse.bacc as bacc
    import concourse.mybir as mybir
    import concourse.tile as tile

    f32 = mybir.dt.float32
    f32r = mybir.dt.float32r
    bf16 = mybir.dt.bfloat16
    f8 = mybir.dt.float8e4
    AX = mybir.AxisListType
    OP = mybir.AluOpType
    AF = mybir.ActivationFunctionType
    DR = mybir.MatmulPerfMode.DoubleRow

    nc = bacc.Bacc(None, target_bir_lowering=False, debug=False)
    xT_d = nc.dram_tensor("xT", [D, TOK], f32r, kind="ExternalInput")
    w_d = nc.dram_tensor("w", [D, E * D], f32r, kind="ExternalInput")
    gw_d = nc.dram_tensor("gate_w", [D, E], f32r, kind="ExternalInput")
    gb_d = nc.dram_tensor("gate_b", [1, E], f32, kind="ExternalInput")
    out_d = nc.dram_tensor("out", [TOK, D], f32, kind="ExternalOutput")

    xT_r = xT_d.rearrange("(k p) t -> k p t", p=P)
    w_r = w_d.rearrange("(k p) (e f) -> k p e f", p=P, f=D)
    gw_r = gw_d.rearrange("(k p) e -> p k e", p=P)
    out_r = out_d.rearrange("(m p) f -> m p f", p=P)

    def v32(ap):  # f32r SBUF data viewed as plain f32 for DVE/ACT reads
        return ap.bitcast(f32)

    with tile.TileContext(nc) as tc:
        with (
            tc.tile_pool(name="persist", bufs=1) as persist,
            tc.tile_pool(name="gstat", bufs=2) as p_gs,
            tc.tile_pool(name="wf", bufs=2 * KT) as p_wf,
            tc.tile_pool(name="tmp", bufs=6) as p_tmp,
            tc.tile_pool(name="ps_gate", bufs=1, space="PSUM") as ps_gate,
            tc.tile_pool(name="ps_cnt", bufs=1, space="PSUM") as ps_cnt,
            tc.tile_pool(name="ps_mm", bufs=6, space="PSUM") as ps_mm,
        ):
            # -- front matter (no DMA dependencies)
            wu_lhs = persist.tile([P, P], bf16, tag="wu_lhs")
            nc.vector.memset(wu_lhs[:], 0.0)
            wu_rhs = persist.tile([P, NH], bf16, tag="wu_rhs")
            nc.vector.memset(wu_rhs[:], 0.0)
            ones_col = persist.tile([P, 1], bf16, tag="ones_col")
            nc.vector.memset(ones_col[:], 1.0)
            exp_in = persist.tile([P, 1], f32, tag="exp_in")
            nc.vector.memset(exp_in[:], 1.0)
            ones_row_bf = persist.tile([1, P], bf16, tag="ones_row_bf")
            nc.vector.memset(ones_row_bf[:], 1.0)
            id8_i = persist.tile([E, E], mybir.dt.int32, tag="id8_i")
            nc.gpsimd.iota(id8_i[:], pattern=[[1, E]], base=0,
                           channel_multiplier=-1)
            id8 = persist.tile([E, E], bf16, tag="id8")
            nc.vector.tensor_scalar(id8[:], id8_i[:], 0, None, op0=OP.is_equal)
            exp_dummy = persist.tile([1, 1], f32, tag="exp_dummy")
            nc.scalar.activation(exp_dummy[:], exp_in[:1, :], AF.Exp)

            # PE warm-up while the first DMAs land
            wu_ps = ps_cnt.tile([P, NH], f32, tag="cnt")
            for i in range(WARMUP_MMS):
                nc.tensor.matmul(
                    wu_ps[:], wu_lhs[:], wu_rhs[:],
                    start=(i == 0), stop=(i == WARMUP_MMS - 1),
                )

            # -- loads: x^T k-tiles interleaved with expert-0 weight k-tiles.
            # fp32 tiles are the PE operands directly (fp32r) -- usable the
            # moment the DMA lands.
            xf = [persist.tile([P, TOK], f32r, tag=f"xf{k}", name=f"xf{k}")
                  for k in range(KT)]
            wf0 = [p_wf.tile([P, D], f32r, tag="wf", name=f"wf0_{k}")
                   for k in range(KT)]
            # The DMA trigger stream is SERIAL (~390 GB/s aggregate, strict
            # issue order), so transfers are ordered by consumption:
            # expert-0 sub-pass operands in k-pair order (half-tile chunks),
            # then the pass-3 weights, then the gating x tail, then the slow
            # strided gate-weight gather, then experts 1..7.
            for k in range(KH):
                for h in range(2):
                    nc.sync.dma_start(xf[k][:, h * NH:(h + 1) * NH],
                                      xT_r[k][:, h * NH:(h + 1) * NH])
                    nc.sync.dma_start(wf0[k][:, h * NH:(h + 1) * NH],
                                      w_r[k, :, 0, h * NH:(h + 1) * NH])
            for k in range(KH, KT):
                nc.sync.dma_start(wf0[k][:], w_r[k, :, 0, :])
            for k in range(KH, KT):
                nc.sync.dma_start(xf[k][:], xT_r[k])

            gb_col = persist.tile([E, 1], f32, tag="gb_col")
            bmb = persist.tile([P, E], f32, tag="bmb")
            gw_in = persist.tile([P, KT, E], f32r, tag="gw_in")
            nc.sync.dma_start(gw_in[:], gw_r[:])
            nc.sync.dma_start(gb_col[:], gb_d.rearrange("o e -> e o"))

            # fp8 operand for the DoubleRow k-pair: x8[:, i, t] = (x/16) for
            # k-tile 6+i, all 1024 tokens. lhsT slices are [p, 2, 128].
            x8 = persist.tile([P, 2, TOK], f8, tag="x8")
            for i in range(2):
                nc.vector.tensor_scalar(x8[:, i, :], v32(xf[6 + i][:]), SX,
                                        None, op0=OP.mult)

            # acc: fp32 output accumulator. acc0/acc1: experts 0/1 computed
            # unscaled (bf16 storage), merged with their gating scales later.
            acc = []
            acc0 = []
            acc1 = []
            for m in range(MT):
                acc.append(persist.tile([P, D], f32, tag=f"acc{m}",
                                        name=f"acc{m}"))
                acc0.append(persist.tile([P, D], bf16, tag=f"acc0{m}",
                                         name=f"acc0{m}"))
                acc1.append(persist.tile([P, D], bf16, tag=f"acc1{m}",
                                         name=f"acc1{m}"))

            fill_ps = ps_gate.tile([P, NH], f32, tag="bmb_ps", bufs=1)

            def pe_filler(n=1):
                for _ in range(n):
                    nc.tensor.matmul(fill_ps[:, :NH], wu_lhs[:], wu_rhs[:],
                                     start=True, stop=True)

            # -- expert 0 in four k-pair sub-passes: each starts as soon as
            # its two k-tiles land (the serial DMA stream would otherwise
            # idle the PE ~9us waiting for all of k0-3).
            def e0_subpass(kp):
                use_dr = (kp == 3 and 0 in FP8_EXPERTS)
                for m in range(MT):
                    ps0 = ps_mm.tile([P, NH], f32, tag="psmm")
                    ps1 = ps_mm.tile([P, NH], f32, tag="psmm")
                    if use_dr:
                        lhs8 = x8[:, :, m * P:(m + 1) * P]
                        nc.tensor.matmul(ps0[:], lhs8, w80[:, :, 0:NH],
                                         start=True, stop=True, perf_mode=DR)
                        nc.tensor.matmul(ps1[:], lhs8, w80[:, :, NH:D],
                                         start=True, stop=True, perf_mode=DR)
                    else:
                        for k in (2 * kp, 2 * kp + 1):
                            lhs = xf[k][:, m * P:(m + 1) * P]
                            nc.tensor.matmul(ps0[:], lhs, wf0[k][:, 0:NH],
                                             start=(k == 2 * kp),
                                             stop=(k == 2 * kp + 1))
                            nc.tensor.matmul(ps1[:], lhs, wf0[k][:, NH:D],
                                             start=(k == 2 * kp),
                                             stop=(k == 2 * kp + 1))
                        if kp == 0 and m < 2:
                            pe_filler(2)
                    for h, ps in ((0, ps0), (1, ps1)):
                        asl = acc0[m][:, h * NH:(h + 1) * NH]
                        if kp == 0:
                            nc.scalar.copy(asl, ps[:])
                        else:
                            nc.vector.tensor_tensor(asl, asl, ps[:],
                                                    op=OP.add)

            e0_subpass(0)
            e0_subpass(1)

            def load_w(e):
                tiles = []
                for k in range(KT):
                    wt = p_wf.tile([P, D], f32r, tag="wf", name=f"wf{e}_{k}")
                    nc.sync.dma_start(wt[:], w_r[k, :, e, :])
                    tiles.append(wt)
                return tiles

            def quant_w(e, wtiles):
                """fp8 W k-pair for expert e: w8[:, i, :] = W[k=6+i]*16."""
                w8 = p_tmp.tile([P, 2, D], f8, tag="w8", name=f"w8_{e}",
                                bufs=2)
                for i in range(2):
                    nc.vector.tensor_scalar(w8[:, i, :],
                                            v32(wtiles[6 + i][:]),
                                            SW, None, op0=OP.mult)
                return w8

            wf_cur = load_w(1)
            w8_cur = quant_w(1, wf_cur) if 1 in FP8_EXPERTS else None
            w80 = quant_w(0, wf0) if 0 in FP8_EXPERTS else None

            e0_subpass(2)
            e0_subpass(3)

            # -- gating logits, TRANSPOSED ([E, tokens]); fp32r from raw x
            lgT0 = ps_mm.tile([E, NH], f32, tag="psmm")
            lgT1 = ps_mm.tile([E, NH], f32, tag="psmm")
            for k in range(KT):
                nc.tensor.matmul(lgT0[:], gw_in[:, k, :], xf[k][:, 0:NH],
                                 start=(k == 0), stop=(k == KT - 1))
                nc.tensor.matmul(lgT1[:], gw_in[:, k, :], xf[k][:, NH:TOK],
                                 start=(k == 0), stop=(k == KT - 1))
            # + gate_b: per-partition bias in the transposed [E, tokens] layout
            lgT_sb = persist.tile([E, TOK], bf16, tag="lgT_sb")
            nc.scalar.activation(lgT_sb[:, 0:NH], lgT0[:], AF.Identity,
                                 bias=gb_col[:])
            nc.scalar.activation(lgT_sb[:, NH:TOK], lgT1[:], AF.Identity,
                                 bias=gb_col[:])

            # Gating part 2: transpose via K=8 matmul against identity, then
            # softmax + top-2 mask; masks packed for a single count matmul.
            mask_all = persist.tile([P, MT * E], bf16, tag="mask_all")
            gfin = []
            gsc = [persist.tile([P, E], f32, tag=f"gsc{m}", name=f"gsc{m}")
                   for m in range(MT)]
            for m in range(MT):
                lg = ps_cnt.tile([P, E], f32, tag="cnt", bufs=1)
                nc.tensor.matmul(lg[:], lgT_sb[:, m * P:(m + 1) * P], id8[:],
                                 start=True, stop=True)
                ex = p_gs.tile([P, E], f32, tag="ex")
                nc.scalar.activation(ex[:], lg[:], AF.Exp)
                ssum = p_gs.tile([P, 1], f32, tag="ssum")
                nc.vector.reduce_sum(ssum[:], ex[:], axis=AX.X)
                rcp = p_gs.tile([P, 1], f32, tag="rcp")
                nc.vector.reciprocal(rcp[:], ssum[:])
                m1 = p_gs.tile([P, 1], f32, tag="m1")
                nc.vector.reduce_max(m1[:], ex[:], axis=AX.X)
                eqb = p_gs.tile([P, E], f32, tag="eqb")
                nc.vector.tensor_scalar(
                    eqb[:], ex[:], m1[:], -1e30, op0=OP.is_ge, op1=OP.mult
                )
                g2 = p_gs.tile([P, E], f32, tag="g2")
                nc.vector.tensor_tensor(g2[:], ex[:], eqb[:], op=OP.add)
                m2 = p_gs.tile([P, 1], f32, tag="m2")
                nc.vector.reduce_max(m2[:], g2[:], axis=AX.X)
                nc.vector.tensor_scalar(mask_all[:, m * E:(m + 1) * E],
                                        ex[:], m2[:], None, op0=OP.is_ge)
                gt = p_gs.tile([P, E], f32, tag=f"gt{m}", bufs=1)
                nc.vector.tensor_scalar_mul(gt[:], ex[:], rcp[:])
                gfin.append(gt)

            cnt_ps = ps_cnt.tile([1, MT * E], f32, tag="cnt")
            nc.tensor.matmul(cnt_ps[:], ones_col[:], mask_all[:],
                             start=True, stop=True)
            cnt_sb = p_gs.tile([1, MT * E], f32, tag="cnt_sb")
            nc.vector.tensor_copy(cnt_sb[:], cnt_ps[:])
            cnt_e = p_gs.tile([1, E], f32, tag="cnt_e")
            nc.vector.tensor_reduce(
                cnt_e[:], cnt_sb[:].rearrange("p (m e) -> p e m", e=E),
                axis=AX.X, op=OP.add,
            )
            bm01 = p_gs.tile([1, E], bf16, tag="bm01")
            nc.vector.tensor_scalar(bm01[:], cnt_e[:], 0.5, None, op0=OP.is_ge)
            bmb_ps = ps_gate.tile([P, E], f32, tag="bmb_ps", bufs=1)
            nc.tensor.matmul(bmb_ps[:], ones_row_bf[:], bm01[:],
                             start=True, stop=True)
            nc.vector.tensor_copy(bmb[:], bmb_ps[:])
            for m in range(MT):
                nc.vector.tensor_tensor(gsc[m][:], gfin[m][:], bmb[:],
                                        op=OP.mult)

            # -- experts 1..7
            for e in range(1, E):
                wfe = wf_cur
                w8e = w8_cur
                if e + 1 < E:
                    wf_cur = load_w(e + 1)
                    w8_cur = (quant_w(e + 1, wf_cur)
                              if (e + 1) in FP8_EXPERTS else None)
                use8 = e in FP8_EXPERTS
                kmax = 6 if use8 else KT
                for m in range(MT):
                    ps0 = ps_mm.tile([P, NH], f32, tag="psmm")
                    ps1 = ps_mm.tile([P, NH], f32, tag="psmm")
                    for k in range(kmax):
                        lhs = xf[k][:, m * P:(m + 1) * P]
                        nc.tensor.matmul(ps0[:], lhs, wfe[k][:, 0:NH],
                                         start=(k == 0),
                                         stop=(not use8 and k == KT - 1))
                        nc.tensor.matmul(ps1[:], lhs, wfe[k][:, NH:D],
                                         start=(k == 0),
                                         stop=(not use8 and k == KT - 1))
                    if use8:
                        lhs8 = x8[:, :, m * P:(m + 1) * P]
                        nc.tensor.matmul(ps0[:], lhs8, w8e[:, :, 0:NH],
                                         start=False, stop=True, perf_mode=DR)
                        nc.tensor.matmul(ps1[:], lhs8, w8e[:, :, NH:D],
                                         start=False, stop=True, perf_mode=DR)
                    for h, ps in ((0, ps0), (1, ps1)):
                        osl = acc[m][:, h * NH:(h + 1) * NH]
                        if e == 1:
                            nc.scalar.copy(acc1[m][:, h * NH:(h + 1) * NH],
                                           ps[:])
                        elif e == 2:
                            nc.scalar.mul(osl, ps[:], gsc[m][:, e:e + 1])
                        else:
                            tmp = p_tmp.tile([P, NH], f32, tag="tmp")
                            nc.scalar.mul(tmp[:], ps[:], gsc[m][:, e:e + 1])
                            nc.vector.tensor_tensor(osl, osl, tmp[:],
                                                    op=OP.add)
                    if e in (3, 4, 5, 6):
                        merge_e = 0 if e in (3, 4) else 1
                        if (e % 2 == 1) == (m < MT // 2):
                            a_un = acc0 if merge_e == 0 else acc1
                            gcol = gsc[m][:, merge_e:merge_e + 1]
                            for h in range(2):
                                osl = acc[m][:, h * NH:(h + 1) * NH]
                                asl = a_un[m][:, h * NH:(h + 1) * NH]
                                tmp = p_tmp.tile([P, NH], f32, tag="tmp")
                                nc.scalar.mul(tmp[:], asl, gcol)
                                nc.vector.tensor_tensor(osl, osl, tmp[:],
                                                        op=OP.add)
                    if e == E - 1:
                        nc.sync.dma_start(out_r[m][:, 0:NH], acc[m][:, 0:NH])
                        nc.sync.dma_start(out_r[m][:, NH:D], acc[m][:, NH:D])

    nc.compile()
    return nc


def _ensure_ntff_hook_module():
    """Defensive: some environments lack ``antenv.axon_hooks``; if a caller
    sets BASS_TRACE=1, run_bass_kernel_spmd imports it unconditionally and
    would crash. Provide a working shim."""
    import sys
    import types

    try:
        import antenv.axon_hooks  # noqa: F401
        return
    except ImportError:
        pass
    try:
        import antenv  # noqa: F401
    except ImportError:
        return
    m = types.ModuleType("antenv.axon_hooks")
    exec(
        "_hook = None\n"
        "def set_axon_ntff_profile_hook(h):\n"
        "    global _hook\n"
        "    _hook = h\n"
        "def get_axon_ntff_profile_hook():\n"
        "    return _hook\n",
        m.__dict__,
    )
    sys.modules["antenv.axon_hooks"] = m
    try:
        from trn_agent_boot.trn_boot import _ntff_profile_via_ctypes

        m.set_axon_ntff_profile_hook(
            _ntff_profile_via_ctypes("/opt/axon/libaxon_pjrt.so")
        )
    except Exception:
        pass


_ensure_ntff_hook_module()

_CACHE = {}
LAST_RESULTS = None  # BassKernelResults of the most recent run (for test.py)


def _get_nc():
    if "nc" not in _CACHE:
        _CACHE["nc"] = _build_nc()
    return _CACHE["nc"]


def kernel(x, W, gate_w, gate_b):
    global LAST_RESULTS
    from concourse.bass_utils import run_bass_kernel_spmd

    x = np.ascontiguousarray(np.asarray(x, dtype=np.float32))
    W = np.ascontiguousarray(np.asarray(W, dtype=np.float32))
    gate_w = np.ascontiguousarray(np.asarray(gate_w, dtype=np.float32))
    gb = np.ascontiguousarray(np.asarray(gate_b, dtype=np.float32).reshape(1, E))

    in_maps = []
    for c in range(N_CORES):
        xT = np.ascontiguousarray(x[c * TOK:(c + 1) * TOK].T)
        in_maps.append({"xT": xT, "w": W, "gate_w": gate_w, "gate_b": gb})

    res = run_bass_kernel_spmd(_get_nc(), in_maps, core_ids=list(range(N_CORES)))
    LAST_RESULTS = res
    return np.concatenate([r["out"] for r in res.results], axis=0)
